# revision 28
# baseline (speedup 1.0000x reference)
"""Trainium2 Bass kernel for nn_CalculateSLayer (GNN message passing).

Computes, for adj (N, N, 2) f32 and s (N, D) f32:
    a     = adj.sum(axis=2)                  # (N, N)
    s_in  = a.T @ s                          # (N, D)
    s_out = a @ s                            # (N, D)
returns (s_in, s_out) — matching the reference's output tuple.

Distribution: adjacency rows sharded across 8 NeuronCores; core c owns
rows I_c = [c*512, (c+1)*512).  From its (512, 4096, 2) block it computes
  * a partial s_in^T (D, N)   = (s[I_c]).T @ a[I_c]   (contracts i)
  * its exact  s_out^T (D,512)  from a[I_c]^T         (contracts j)
Host sums the 8 s_in partials and concatenates the s_out blocks.

v2 (bf16): the kernel runs in bf16 (measured rel L2 error 2.6e-3 vs the
f32 reference — the threshold is 2e-2).  This halves the HBM stream
(16.8 -> 8.4 MB/core) and doubles PE matmul column rate.

Per-core dataflow:
  host: cast adj block to bf16 and relayout to [jc][p][k][it][j] so each
        chunk is one fully contiguous 1 MB DMA with 8 KB/partition lines
        (k-planes de-interleaved for contiguous DVE adds).  The last
        chunk is split into two 256-column subchunks to shorten the
        post-last-byte tail.
  DMA : 9 chunk loads issued up front on the sync HWDGE queue; small
        s/identity loads ride the gpsimd queue in parallel.
  DVE : channel add a_ch = k0 + k1 (bf16, contiguous), psum evacuations.
  PE  : ~20 warm-up matmuls on a zero tile defeat the HAM cold clock
        (1.2 GHz) before real work;
        pair-transposes: a_ch viewed as f32 packs two bf16 j's per
        element, so 128x128 f32 exact-permutation transposes move two
        j-columns at once (64 transposes instead of 128);
        s_in  matmul psum_sin(70,512) += s_own[it].T @ a_ch
        s_out matmul psum_out(70,512) += s_perm[jc,t,r].T @ aT[t][:,:,r]
        (aT viewed as [128, 512, 2] bf16; r indexes the packed pair),
        pipelined one chunk behind the transposes.
  ACT : psT -> aT SBUF evacuation (f32 bit-preserving).
  DMA : s_in^T quarter flushes (bf16) mid-stream, s_out^T at the end.
"""

import numpy as np
import ml_dtypes

import concourse.bass as bass
from concourse import bacc
import concourse.mybir as mybir
import concourse.tile as tile
from concourse import bass_utils

N = 4096          # nodes
D = 70            # embedding dim
NCORES = 8
RB = N // NCORES  # 512 rows per core
P = 128           # partitions
IT = RB // P      # 4 i-tiles per core
WJ = 512          # j-chunk width
JC = N // WJ      # 8 j-chunks
HJ = WJ // 2      # subchunk width for the last chunk
N_WARM = 15       # PE warm-up matmuls (HAM un-throttle)

F32 = mybir.dt.float32
F32R = mybir.dt.float32r
BF16 = mybir.dt.bfloat16
F8 = mybir.dt.float8e4

# Set by the test harness to capture a profile; the grading path leaves these
# untouched.
TRACE = False
TRACE_KWARGS = {}
LAST_RESULT = None


def _emit(nc: bass.Bass, adjq, s_own_q, s_perm_q, s_inT, s_outT):
    with tile.TileContext(nc) as tc:
        with (
            tc.tile_pool(name="raw", bufs=JC) as raw_pool,
            tc.tile_pool(name="work", bufs=1) as work,
            tc.tile_pool(name="singles", bufs=1) as singles,
            tc.tile_pool(name="psT", bufs=1, space="PSUM") as psT_pool,
            tc.tile_pool(name="psSin", bufs=1, space="PSUM") as psSin_pool,
            tc.tile_pool(name="psOut", bufs=1, space="PSUM") as psOut_pool,
            tc.tile_pool(name="psWarm", bufs=1, space="PSUM") as psWarm_pool,
        ):
            # ---- persistent tiles / gpsimd-side small loads -----------------
            wtile = singles.tile([P, 640], BF16, name="wtile")
            nc.gpsimd.memset(wtile, 0)

            # ---- input DMAs -------------------------------------------------
            # 8 fp8 chunk loads (512 KB each, 4 KB/partition contiguous lines)
            # on the sync HWDGE queue, issued up front
            raws = []
            s_perm_sb = singles.tile([P, JC * 2 * 2, D], BF16)
            for sc in range(JC):
                r = raw_pool.tile([P, 2, IT, WJ], F8, tag="raw")
                nc.sync.dma_start(out=r, in_=adjq[sc])
                raws.append(r)
                if sc == 0:
                    # small loads ride the gpsimd queue in parallel
                    ident_dram = nc.inline_tensor(
                        np.eye(P, dtype=np.float32), name="ident_const"
                    )
                    ident = singles.tile([P, P], F32)
                    nc.gpsimd.dma_start(out=ident, in_=ident_dram.ap())
                    s_own_sb = singles.tile([P, IT, D], BF16)
                    nc.gpsimd.dma_start(out=s_own_sb, in_=s_own_q)
                elif sc == 1:
                    # s_perm on the FAST sync queue right behind chunk 1: it
                    # must land before the first s_out matmuls (~chunk 1's
                    # processing); the gpsimd software queue is far too slow
                    # for 560KB (observed landing at ~20us, stalling the PE)
                    nc.sync.dma_start(out=s_perm_sb, in_=s_perm_q)

            # ring of 3: the DVE adds run ahead of the PE so a PE hiccup
            # never starves the next chunk's add
            a_chs = [
                work.tile([P, IT, WJ], BF16, name=f"a_ch_{par}") for par in range(3)
            ]
            # aT[t] viewed two ways: f32 (evac), [512,2] bf16 (matmul rhs);
            # ring of 3 so the ACT evac of chunk jc+2 never collides with the
            # PE still reading chunk jc's tiles
            aTs = [
                [work.tile([P, WJ, 2], BF16, name=f"aT_{par}_{t}") for t in range(2)]
                for par in range(3)
            ]
            sin_sb = work.tile([D, N], BF16, name="sin_sb")
            sout_sb = work.tile([D, RB], BF16, name="sout_sb")

            psT = [
                [psT_pool.tile([P, RB], F32, name=f"psT_{par}_{t}") for t in range(2)]
                for par in range(2)
            ]
            psum_sins = [
                psSin_pool.tile([D, WJ], F32, name=f"psum_sin_{par}")
                for par in range(2)
            ]
            psum_sins.append(psWarm_pool.tile([D, WJ], F32, name="psum_sin_2"))
            psum_out = psOut_pool.tile([D, RB], F32)

            # ---- PE warm-up: defeat the HAM cold clock ----------------------
            # (writes garbage into sin bank 2, first really used at chunk 2)
            for _ in range(N_WARM):
                nc.tensor.matmul(
                    psum_sins[2], lhsT=wtile[:, :D], rhs=wtile[:, P:P + WJ],
                    start=True, stop=True,
                )

            def emit_sout_mms(jc, t):
                """s_out accumulation for chunk jc, pair-block t (aT already
                evacuated; runs one chunk behind the transposes)."""
                aT_b = aTs[jc % 3][t]
                for r in range(2):
                    k = (jc * 2 + t) * 2 + r
                    nc.tensor.matmul(
                        psum_out,
                        lhsT=s_perm_sb[:, k, :],
                        rhs=aT_b[:, :, r],
                        start=(k == 0),
                        stop=(k == 2 * 2 * JC - 1),
                    )

            def emit_filler(n):
                """Keep the PE's HAM activity window busy across early data
                gaps (idle > ~3.4us re-throttles the PE clock to 1.2 GHz)."""
                for _ in range(n):
                    nc.tensor.matmul(
                        psum_sins[2][:, :P], lhsT=wtile[:, :D], rhs=wtile[:, P:2 * P],
                        start=True, stop=True,
                    )

            def emit_sin_cast(jc, engine):
                """psum_sin -> sin_sb staging (on the scalar engine mid-stream:
                the DVE must stay add-only or the in-order DVE queue couples
                the adds to PE progress and stretches the whole pipeline),
                plus the quarter flush once both covered chunks are staged."""
                if engine == "scalar":
                    nc.scalar.copy(
                        out=sin_sb[:, jc * WJ:(jc + 1) * WJ], in_=psum_sins[jc % 3]
                    )
                else:
                    nc.vector.tensor_copy(
                        out=sin_sb[:, jc * WJ:(jc + 1) * WJ], in_=psum_sins[jc % 3]
                    )
                if jc % 2 == 1:
                    q = jc // 2
                    nc.sync.dma_start(
                        out=s_inT[q], in_=sin_sb[:, q * (N // 4):(q + 1) * (N // 4)]
                    )

            # ---- main loop over fp8 chunks ----------------------------------
            for jc in range(JC):
                par = jc % 3
                a_ch = a_chs[par]
                a_f32 = a_ch.bitcast(F32)  # [P, IT, WJ//2] packed bf16 pairs
                # channel add: fp8 planes in, bf16 out (upconvert on the DVE);
                # per-it pieces for chunk 0 so the first transposes start
                # ~1.7us earlier (Tile tracks subtile ranges)
                if jc == 0:
                    for it in range(IT):
                        eng = nc.vector if it < 2 else nc.gpsimd
                        eng.tensor_add(
                            out=a_ch[:, it, :],
                            in0=raws[jc][:, 0, it, :],
                            in1=raws[jc][:, 1, it, :],
                        )
                else:
                    nc.vector.tensor_add(
                        out=a_ch[:, 0:2], in0=raws[jc][:, 0, 0:2],
                        in1=raws[jc][:, 1, 0:2],
                    )
                    nc.gpsimd.tensor_add(
                        out=a_ch[:, 2:4], in0=raws[jc][:, 0, 2:4],
                        in1=raws[jc][:, 1, 2:4],
                    )
                for t in range(2):
                    for it in range(IT):
                        nc.tensor.transpose(
                            psT[jc % 2][t][:, it * P:(it + 1) * P],
                            a_f32[:, it, t * P:(t + 1) * P],
                            ident,
                        )
                    if jc > 0:
                        emit_sout_mms(jc - 1, t)
                    nc.scalar.copy(
                        out=aTs[jc % 3][t].bitcast(F32), in_=psT[jc % 2][t]
                    )
                for it in range(IT):
                    nc.tensor.matmul(
                        psum_sins[jc % 3],
                        lhsT=s_own_sb[:, it, :],
                        rhs=a_ch[:, it, :],
                        start=(it == 0),
                        stop=(it == IT - 1),
                    )
                if jc > 0:
                    # after this chunk's aT evacs in the scalar queue, so the
                    # evacs (which gate next chunk's s_out) are never delayed
                    emit_sin_cast(jc - 1, "scalar")
                if jc < 2:
                    emit_filler(4)

            # ---- epilogue: last chunk's own s_out, casts, final flushes -----
            jc = JC - 1
            emit_sout_mms(jc, 0)
            emit_sout_mms(jc, 1)
            emit_sin_cast(jc, "vector")
            nc.scalar.copy(out=sout_sb, in_=psum_out)
            # last flush on the idle sync HWDGE queue, parallel to the gpsimd one
            nc.sync.dma_start(out=s_outT, in_=sout_sb)


_ENGINE_SEM_PREFIX = {
    "PE": "PE_",
    "DVE": "DVE_",
    "Activation": "Activation_",
    "Pool": "Pool_",
    "SP": "SP_",
}

_SKIP_OPS = ("InstEventSemaphore", "InstDrain", "InstDMACopy", "InstBranch")


def _strip_self_waits(nc: bass.Bass) -> int:
    """Drop semaphore waits where an instruction waits on its OWN engine's
    completion semaphore.  Engine queues issue and complete in order, so such
    waits are always runtime-satisfied; Tile emits them anyway and they push
    instructions past walrus codegen's per-opcode sync-wait limits (most
    compute encodings accept a single wait)."""
    stripped = 0
    for _, inst in nc.inst_map.items():
        if type(inst).__name__ in _SKIP_OPS:
            continue
        si = getattr(inst, "sync_info", None)
        if si is None or not si.on_wait:
            continue
        eng = getattr(inst, "engine", None)
        prefix = _ENGINE_SEM_PREFIX.get(getattr(eng, "name", ""), None)
        if prefix is None:
            continue
        kept = [w for w in si.on_wait if not w.ant_name.startswith(prefix)]
        if len(kept) != len(si.on_wait):
            stripped += len(si.on_wait) - len(kept)
            si.on_wait = kept
    return stripped


def _build() -> bass.Bass:
    nc = bacc.Bacc("TRN2", num_devices=NCORES)
    adjq = nc.dram_tensor("adjq", [JC, P, 2, IT, WJ], F8, kind="ExternalInput")
    s_own_q = nc.dram_tensor("s_own_q", [P, IT, D], BF16, kind="ExternalInput")
    s_perm_q = nc.dram_tensor("s_perm_q", [P, JC * 2 * 2, D], BF16, kind="ExternalInput")
    s_inT = [
        nc.dram_tensor(f"s_inT_{h}", [D, N // 4], BF16, kind="ExternalOutput")
        for h in range(4)
    ]
    s_outT = nc.dram_tensor("s_outT", [D, RB], BF16, kind="ExternalOutput")
    _emit(
        nc,
        adjq.ap(),
        s_own_q.ap(),
        s_perm_q.ap(),
        [t.ap() for t in s_inT],
        s_outT.ap(),
    )
    _strip_self_waits(nc)
    nc.finalize()
    return nc


_nc_cache = None


def _prep_core_inputs(adj_bf, s_bf, c):
    """Host-side relayout of core c's adjacency block and s tiles."""
    blk = adj_bf[c * RB:(c + 1) * RB]                    # (512, 4096, 2) fp8
    v = blk.reshape(IT, P, JC, WJ, 2)                     # it, p, jc, j, k
    v = v.transpose(2, 1, 4, 0, 3)                        # jc, p, k, it, j
    adjq = np.ascontiguousarray(v)
    s_own_q = np.ascontiguousarray(
        s_bf[c * RB:(c + 1) * RB].reshape(IT, P, D).transpose(1, 0, 2)
    )
    return {"adjq": adjq, "s_own_q": s_own_q}


def kernel(adj: np.ndarray, s: np.ndarray):
    global _nc_cache, LAST_RESULT
    adj = np.asarray(adj)
    s = np.asarray(s)
    assert adj.shape == (N, N, 2) and s.shape == (N, D)

    if _nc_cache is None:
        _nc_cache = _build()
    nc = _nc_cache

    # centered fp8 quantization: adj = (adj - 0.5) in e4m3 + exact rank-1
    # correction (sum_k 0.5 * colsum(s)) applied on the host after gather
    adj_bf = (np.asarray(adj, np.float32) - np.float32(0.5)).astype(
        ml_dtypes.float8_e4m3
    )
    s_bf = np.asarray(s, np.float32).astype(ml_dtypes.bfloat16)
    csum = np.asarray(s, np.float64).sum(axis=0)
    # s_perm[p, (jc, t, r)] = s[jc*512 + (t*128 + p)*2 + r]   (partition-major)
    s_perm = np.ascontiguousarray(
        s_bf.reshape(JC, 2, P, 2, D).transpose(2, 0, 1, 3, 4).reshape(P, JC * 4, D)
    )

    in_maps = []
    for c in range(NCORES):
        m = _prep_core_inputs(adj_bf, s_bf, c)
        m["s_perm_q"] = s_perm
        in_maps.append(m)

    res = bass_utils.run_bass_kernel_spmd(
        nc,
        in_maps,
        core_ids=list(range(NCORES)),
        trace=TRACE,
        **TRACE_KWARGS,
    )
    LAST_RESULT = res

    s_in = (
        (
            np.sum(
                [
                    np.concatenate(
                        [np.asarray(r[f"s_inT_{h}"], np.float32) for h in range(4)],
                        axis=1,
                    )
                    for r in res.results
                ],
                axis=0,
                dtype=np.float64,
            ).T
            + csum[None, :]
        )
        .astype(np.float32)
    )
    s_out = (
        np.concatenate(
            [np.asarray(r["s_outT"], np.float64).T for r in res.results], axis=0
        )
        + csum[None, :]
    ).astype(np.float32)
    return (np.ascontiguousarray(s_in), np.ascontiguousarray(s_out))


# revision 29
# speedup vs baseline: 1.0482x; 1.0482x over previous
"""Trainium2 Bass kernel for nn_CalculateSLayer (GNN message passing).

Computes, for adj (N, N, 2) f32 and s (N, D) f32:
    a     = adj.sum(axis=2)                  # (N, N)
    s_in  = a.T @ s                          # (N, D)
    s_out = a @ s                            # (N, D)
returns (s_in, s_out) — matching the reference's output tuple.

Distribution: adjacency rows sharded across 8 NeuronCores; core c owns
rows I_c = [c*512, (c+1)*512).  From its (512, 4096, 2) block it computes
  * a partial s_in^T (D, N)   = (s[I_c]).T @ a[I_c]   (contracts i)
  * its exact  s_out^T (D,512)  from a[I_c]^T         (contracts j)
Host sums the 8 s_in partials and concatenates the s_out blocks.

v2 (bf16): the kernel runs in bf16 (measured rel L2 error 2.6e-3 vs the
f32 reference — the threshold is 2e-2).  This halves the HBM stream
(16.8 -> 8.4 MB/core) and doubles PE matmul column rate.

Per-core dataflow:
  host: cast adj block to bf16 and relayout to [jc][p][k][it][j] so each
        chunk is one fully contiguous 1 MB DMA with 8 KB/partition lines
        (k-planes de-interleaved for contiguous DVE adds).  The last
        chunk is split into two 256-column subchunks to shorten the
        post-last-byte tail.
  DMA : 9 chunk loads issued up front on the sync HWDGE queue; small
        s/identity loads ride the gpsimd queue in parallel.
  DVE : channel add a_ch = k0 + k1 (bf16, contiguous), psum evacuations.
  PE  : ~20 warm-up matmuls on a zero tile defeat the HAM cold clock
        (1.2 GHz) before real work;
        pair-transposes: a_ch viewed as f32 packs two bf16 j's per
        element, so 128x128 f32 exact-permutation transposes move two
        j-columns at once (64 transposes instead of 128);
        s_in  matmul psum_sin(70,512) += s_own[it].T @ a_ch
        s_out matmul psum_out(70,512) += s_perm[jc,t,r].T @ aT[t][:,:,r]
        (aT viewed as [128, 512, 2] bf16; r indexes the packed pair),
        pipelined one chunk behind the transposes.
  ACT : psT -> aT SBUF evacuation (f32 bit-preserving).
  DMA : s_in^T quarter flushes (bf16) mid-stream, s_out^T at the end.
"""

import numpy as np
import ml_dtypes

import concourse.bass as bass
from concourse import bacc
import concourse.mybir as mybir
import concourse.tile as tile
from concourse import bass_utils

N = 4096          # nodes
D = 70            # embedding dim
NCORES = 8
RB = N // NCORES  # 512 rows per core
P = 128           # partitions
IT = RB // P      # 4 i-tiles per core
WJ = 512          # j-chunk width
JC = N // WJ      # 8 j-chunks
HJ = WJ // 2      # subchunk width for the last chunk
N_WARM = 15       # PE warm-up matmuls (HAM un-throttle)

F32 = mybir.dt.float32
F32R = mybir.dt.float32r
BF16 = mybir.dt.bfloat16
F8 = mybir.dt.float8e4

# Set by the test harness to capture a profile; the grading path leaves these
# untouched.
TRACE = False
TRACE_KWARGS = {}
LAST_RESULT = None


def _emit(nc: bass.Bass, adjq, s_own_q, s_perm_q, s_inT, s_outT):
    with tile.TileContext(nc) as tc:
        with (
            tc.tile_pool(name="raw", bufs=JC) as raw_pool,
            tc.tile_pool(name="work", bufs=1) as work,
            tc.tile_pool(name="singles", bufs=1) as singles,
            tc.tile_pool(name="psT", bufs=1, space="PSUM") as psT_pool,
            tc.tile_pool(name="psSin", bufs=1, space="PSUM") as psSin_pool,
            tc.tile_pool(name="psOut", bufs=1, space="PSUM") as psOut_pool,
            tc.tile_pool(name="psWarm", bufs=1, space="PSUM") as psWarm_pool,
        ):
            # ---- persistent tiles / gpsimd-side small loads -----------------
            wtile = singles.tile([P, 640], BF16, name="wtile")
            nc.gpsimd.memset(wtile, 0)

            # ---- input DMAs -------------------------------------------------
            # 8 fp8 chunk loads (512 KB each, 4 KB/partition contiguous lines)
            # on the sync HWDGE queue, issued up front
            raws = []
            s_perm_sb = singles.tile([P, JC * 2 * 2, D], BF16)
            for sc in range(JC):
                r = raw_pool.tile([P, 2, IT, WJ], F8, tag="raw")
                nc.sync.dma_start(out=r, in_=adjq[sc])
                raws.append(r)
                if sc == 0:
                    # small loads ride the gpsimd queue in parallel
                    ident_dram = nc.inline_tensor(
                        np.eye(P, dtype=np.float32), name="ident_const"
                    )
                    ident = singles.tile([P, P], F32)
                    nc.gpsimd.dma_start(out=ident, in_=ident_dram.ap())
                    s_own_sb = singles.tile([P, IT, D], BF16)
                    nc.gpsimd.dma_start(out=s_own_sb, in_=s_own_q)
                elif sc == 1:
                    # s_perm on the FAST sync queue right behind chunk 1: it
                    # must land before the first s_out matmuls (~chunk 1's
                    # processing); the gpsimd software queue is far too slow
                    # for 560KB (observed landing at ~20us, stalling the PE)
                    nc.sync.dma_start(out=s_perm_sb, in_=s_perm_q)

            # ring of 3: the DVE adds run ahead of the PE so a PE hiccup
            # never starves the next chunk's add
            a_chs = [
                work.tile([P, IT, WJ], BF16, name=f"a_ch_{par}") for par in range(3)
            ]
            # aT[t] viewed two ways: f32 (evac), [512,2] bf16 (matmul rhs);
            # ring of 3 so the ACT evac of chunk jc+2 never collides with the
            # PE still reading chunk jc's tiles
            aTs = [
                [work.tile([P, WJ, 2], BF16, name=f"aT_{par}_{t}") for t in range(2)]
                for par in range(3)
            ]
            sin_sb = work.tile([D, N], BF16, name="sin_sb")
            sout_sb = work.tile([D, RB], BF16, name="sout_sb")

            psT = [
                [psT_pool.tile([P, RB], F32, name=f"psT_{par}_{t}") for t in range(2)]
                for par in range(2)
            ]
            psum_sins = [
                psSin_pool.tile([D, WJ], F32, name=f"psum_sin_{par}")
                for par in range(2)
            ]
            psum_sins.append(psWarm_pool.tile([D, WJ], F32, name="psum_sin_2"))
            psum_out = psOut_pool.tile([D, RB], F32)

            # ---- PE warm-up: defeat the HAM cold clock ----------------------
            # (writes garbage into sin bank 2, first really used at chunk 2)
            for _ in range(N_WARM):
                nc.tensor.matmul(
                    psum_sins[2], lhsT=wtile[:, :D], rhs=wtile[:, P:P + WJ],
                    start=True, stop=True,
                )

            def emit_sout_mms(jc, t):
                """s_out accumulation for chunk jc, pair-block t (aT already
                evacuated; runs one chunk behind the transposes)."""
                aT_b = aTs[jc % 3][t]
                for r in range(2):
                    k = (jc * 2 + t) * 2 + r
                    nc.tensor.matmul(
                        psum_out,
                        lhsT=s_perm_sb[:, k, :],
                        rhs=aT_b[:, :, r],
                        start=(k == 0),
                        stop=(k == 2 * 2 * JC - 1),
                    )

            def emit_filler(n):
                """Keep the PE's HAM activity window busy across early data
                gaps (idle > ~3.4us re-throttles the PE clock to 1.2 GHz)."""
                for _ in range(n):
                    nc.tensor.matmul(
                        psum_sins[2][:, :P], lhsT=wtile[:, :D], rhs=wtile[:, P:2 * P],
                        start=True, stop=True,
                    )

            def emit_sin_cast(jc, engine):
                """psum_sin -> sin_sb staging (on the scalar engine mid-stream:
                the DVE must stay add-only or the in-order DVE queue couples
                the adds to PE progress and stretches the whole pipeline),
                plus the quarter flush once both covered chunks are staged."""
                if engine == "scalar":
                    nc.scalar.copy(
                        out=sin_sb[:, jc * WJ:(jc + 1) * WJ], in_=psum_sins[jc % 3]
                    )
                else:
                    nc.vector.tensor_copy(
                        out=sin_sb[:, jc * WJ:(jc + 1) * WJ], in_=psum_sins[jc % 3]
                    )
                if jc % 2 == 1:
                    q = jc // 2
                    nc.sync.dma_start(
                        out=s_inT[q], in_=sin_sb[:, q * (N // 4):(q + 1) * (N // 4)]
                    )

            # ---- main loop over fp8 chunks ----------------------------------
            for jc in range(JC):
                par = jc % 3
                a_ch = a_chs[par]
                a_f32 = a_ch.bitcast(F32)  # [P, IT, WJ//2] packed bf16 pairs
                # channel add: fp8 planes in, bf16 out (upconvert on the DVE);
                # per-it pieces for chunk 0 so the first transposes start
                # ~1.7us earlier (Tile tracks subtile ranges)
                if jc == 0:
                    for it in range(IT):
                        nc.vector.tensor_add(
                            out=a_ch[:, it, :],
                            in0=raws[jc][:, 0, it, :],
                            in1=raws[jc][:, 1, it, :],
                        )
                else:
                    nc.vector.tensor_add(
                        out=a_ch, in0=raws[jc][:, 0], in1=raws[jc][:, 1]
                    )
                for t in range(2):
                    for it in range(IT):
                        nc.tensor.transpose(
                            psT[jc % 2][t][:, it * P:(it + 1) * P],
                            a_f32[:, it, t * P:(t + 1) * P],
                            ident,
                        )
                    if jc > 0:
                        emit_sout_mms(jc - 1, t)
                    nc.scalar.copy(
                        out=aTs[jc % 3][t].bitcast(F32), in_=psT[jc % 2][t]
                    )
                for it in range(IT):
                    nc.tensor.matmul(
                        psum_sins[jc % 3],
                        lhsT=s_own_sb[:, it, :],
                        rhs=a_ch[:, it, :],
                        start=(it == 0),
                        stop=(it == IT - 1),
                    )
                if jc > 0:
                    # after this chunk's aT evacs in the scalar queue, so the
                    # evacs (which gate next chunk's s_out) are never delayed
                    emit_sin_cast(jc - 1, "scalar")
                if jc < 2:
                    emit_filler(4)

            # ---- epilogue: last chunk's own s_out, casts, final flushes -----
            jc = JC - 1
            emit_sout_mms(jc, 0)
            emit_sout_mms(jc, 1)
            emit_sin_cast(jc, "vector")
            nc.scalar.copy(out=sout_sb, in_=psum_out)
            # last flush on the idle sync HWDGE queue, parallel to the gpsimd one
            nc.sync.dma_start(out=s_outT, in_=sout_sb)


_ENGINE_SEM_PREFIX = {
    "PE": "PE_",
    "DVE": "DVE_",
    "Activation": "Activation_",
    "Pool": "Pool_",
    "SP": "SP_",
}

_SKIP_OPS = ("InstEventSemaphore", "InstDrain", "InstDMACopy", "InstBranch")


def _strip_self_waits(nc: bass.Bass) -> int:
    """Drop semaphore waits where an instruction waits on its OWN engine's
    completion semaphore.  Engine queues issue and complete in order, so such
    waits are always runtime-satisfied; Tile emits them anyway and they push
    instructions past walrus codegen's per-opcode sync-wait limits (most
    compute encodings accept a single wait)."""
    stripped = 0
    for _, inst in nc.inst_map.items():
        if type(inst).__name__ in _SKIP_OPS:
            continue
        si = getattr(inst, "sync_info", None)
        if si is None or not si.on_wait:
            continue
        eng = getattr(inst, "engine", None)
        prefix = _ENGINE_SEM_PREFIX.get(getattr(eng, "name", ""), None)
        if prefix is None:
            continue
        kept = [w for w in si.on_wait if not w.ant_name.startswith(prefix)]
        if len(kept) != len(si.on_wait):
            stripped += len(si.on_wait) - len(kept)
            si.on_wait = kept
    return stripped


def _build() -> bass.Bass:
    nc = bacc.Bacc("TRN2", num_devices=NCORES)
    adjq = nc.dram_tensor("adjq", [JC, P, 2, IT, WJ], F8, kind="ExternalInput")
    s_own_q = nc.dram_tensor("s_own_q", [P, IT, D], BF16, kind="ExternalInput")
    s_perm_q = nc.dram_tensor("s_perm_q", [P, JC * 2 * 2, D], BF16, kind="ExternalInput")
    s_inT = [
        nc.dram_tensor(f"s_inT_{h}", [D, N // 4], BF16, kind="ExternalOutput")
        for h in range(4)
    ]
    s_outT = nc.dram_tensor("s_outT", [D, RB], BF16, kind="ExternalOutput")
    _emit(
        nc,
        adjq.ap(),
        s_own_q.ap(),
        s_perm_q.ap(),
        [t.ap() for t in s_inT],
        s_outT.ap(),
    )
    _strip_self_waits(nc)
    nc.finalize()
    return nc


_nc_cache = None


def _prep_core_inputs(adj_bf, s_bf, c):
    """Host-side relayout of core c's adjacency block and s tiles."""
    blk = adj_bf[c * RB:(c + 1) * RB]                    # (512, 4096, 2) fp8
    v = blk.reshape(IT, P, JC, WJ, 2)                     # it, p, jc, j, k
    v = v.transpose(2, 1, 4, 0, 3)                        # jc, p, k, it, j
    adjq = np.ascontiguousarray(v)
    s_own_q = np.ascontiguousarray(
        s_bf[c * RB:(c + 1) * RB].reshape(IT, P, D).transpose(1, 0, 2)
    )
    return {"adjq": adjq, "s_own_q": s_own_q}


def kernel(adj: np.ndarray, s: np.ndarray):
    global _nc_cache, LAST_RESULT
    adj = np.asarray(adj)
    s = np.asarray(s)
    assert adj.shape == (N, N, 2) and s.shape == (N, D)

    if _nc_cache is None:
        _nc_cache = _build()
    nc = _nc_cache

    # centered fp8 quantization: adj = (adj - 0.5) in e4m3 + exact rank-1
    # correction (sum_k 0.5 * colsum(s)) applied on the host after gather
    adj_bf = (np.asarray(adj, np.float32) - np.float32(0.5)).astype(
        ml_dtypes.float8_e4m3
    )
    s_bf = np.asarray(s, np.float32).astype(ml_dtypes.bfloat16)
    csum = np.asarray(s, np.float64).sum(axis=0)
    # s_perm[p, (jc, t, r)] = s[jc*512 + (t*128 + p)*2 + r]   (partition-major)
    s_perm = np.ascontiguousarray(
        s_bf.reshape(JC, 2, P, 2, D).transpose(2, 0, 1, 3, 4).reshape(P, JC * 4, D)
    )

    in_maps = []
    for c in range(NCORES):
        m = _prep_core_inputs(adj_bf, s_bf, c)
        m["s_perm_q"] = s_perm
        in_maps.append(m)

    res = bass_utils.run_bass_kernel_spmd(
        nc,
        in_maps,
        core_ids=list(range(NCORES)),
        trace=TRACE,
        **TRACE_KWARGS,
    )
    LAST_RESULT = res

    s_in = (
        (
            np.sum(
                [
                    np.concatenate(
                        [np.asarray(r[f"s_inT_{h}"], np.float32) for h in range(4)],
                        axis=1,
                    )
                    for r in res.results
                ],
                axis=0,
                dtype=np.float64,
            ).T
            + csum[None, :]
        )
        .astype(np.float32)
    )
    s_out = (
        np.concatenate(
            [np.asarray(r["s_outT"], np.float64).T for r in res.results], axis=0
        )
        + csum[None, :]
    ).astype(np.float32)
    return (np.ascontiguousarray(s_in), np.ascontiguousarray(s_out))


# revision 30
# speedup vs baseline: 1.0763x; 1.0268x over previous
"""Trainium2 Bass kernel for nn_CalculateSLayer (GNN message passing).

Computes, for adj (N, N, 2) f32 and s (N, D) f32:
    a     = adj.sum(axis=2)                  # (N, N)
    s_in  = a.T @ s                          # (N, D)
    s_out = a @ s                            # (N, D)
returns (s_in, s_out) — matching the reference's output tuple.

Distribution: adjacency rows sharded across 8 NeuronCores; core c owns
rows I_c = [c*512, (c+1)*512).  From its (512, 4096, 2) block it computes
  * a partial s_in^T (D, N)   = (s[I_c]).T @ a[I_c]   (contracts i)
  * its exact  s_out^T (D,512)  from a[I_c]^T         (contracts j)
Host sums the 8 s_in partials and concatenates the s_out blocks.

v2 (bf16): the kernel runs in bf16 (measured rel L2 error 2.6e-3 vs the
f32 reference — the threshold is 2e-2).  This halves the HBM stream
(16.8 -> 8.4 MB/core) and doubles PE matmul column rate.

Per-core dataflow:
  host: cast adj block to bf16 and relayout to [jc][p][k][it][j] so each
        chunk is one fully contiguous 1 MB DMA with 8 KB/partition lines
        (k-planes de-interleaved for contiguous DVE adds).  The last
        chunk is split into two 256-column subchunks to shorten the
        post-last-byte tail.
  DMA : 9 chunk loads issued up front on the sync HWDGE queue; small
        s/identity loads ride the gpsimd queue in parallel.
  DVE : channel add a_ch = k0 + k1 (bf16, contiguous), psum evacuations.
  PE  : ~20 warm-up matmuls on a zero tile defeat the HAM cold clock
        (1.2 GHz) before real work;
        pair-transposes: a_ch viewed as f32 packs two bf16 j's per
        element, so 128x128 f32 exact-permutation transposes move two
        j-columns at once (64 transposes instead of 128);
        s_in  matmul psum_sin(70,512) += s_own[it].T @ a_ch
        s_out matmul psum_out(70,512) += s_perm[jc,t,r].T @ aT[t][:,:,r]
        (aT viewed as [128, 512, 2] bf16; r indexes the packed pair),
        pipelined one chunk behind the transposes.
  ACT : psT -> aT SBUF evacuation (f32 bit-preserving).
  DMA : s_in^T quarter flushes (bf16) mid-stream, s_out^T at the end.
"""

import numpy as np
import ml_dtypes

import concourse.bass as bass
from concourse import bacc
import concourse.mybir as mybir
import concourse.tile as tile
from concourse import bass_utils

N = 4096          # nodes
D = 70            # embedding dim
NCORES = 8
RB = N // NCORES  # 512 rows per core
P = 128           # partitions
IT = RB // P      # 4 i-tiles per core
WJ = 512          # j-chunk width
JC = N // WJ      # 8 j-chunks
HJ = WJ // 2      # subchunk width for the last chunk
N_WARM = 15       # PE warm-up matmuls (HAM un-throttle)

F32 = mybir.dt.float32
F32R = mybir.dt.float32r
BF16 = mybir.dt.bfloat16
F8 = mybir.dt.float8e4

# Set by the test harness to capture a profile; the grading path leaves these
# untouched.
TRACE = False
TRACE_KWARGS = {}
LAST_RESULT = None


def _emit(nc: bass.Bass, adjq, s_own_q, s_perm_q, s_inT, s_outT):
    with tile.TileContext(nc) as tc:
        with (
            tc.tile_pool(name="raw", bufs=JC) as raw_pool,
            tc.tile_pool(name="work", bufs=1) as work,
            tc.tile_pool(name="singles", bufs=1) as singles,
            tc.tile_pool(name="psT", bufs=1, space="PSUM") as psT_pool,
            tc.tile_pool(name="psSin", bufs=1, space="PSUM") as psSin_pool,
            tc.tile_pool(name="psOut", bufs=1, space="PSUM") as psOut_pool,
            tc.tile_pool(name="psWarm", bufs=1, space="PSUM") as psWarm_pool,
        ):
            # ---- persistent tiles / gpsimd-side small loads -----------------
            wtile = singles.tile([P, 640], BF16, name="wtile")
            nc.gpsimd.memset(wtile, 0)

            # ---- input DMAs -------------------------------------------------
            # 8 fp8 chunk loads (512 KB each, 4 KB/partition contiguous lines)
            # on the sync HWDGE queue, issued up front
            raws = []
            s_perm_sb = singles.tile([P, JC * 2 * 2, D], BF16)
            for sc in range(JC):
                r = raw_pool.tile([P, 2, IT, WJ], F8, tag="raw")
                nc.sync.dma_start(out=r, in_=adjq[sc])
                raws.append(r)
                if sc == 0:
                    # small loads ride the gpsimd queue in parallel
                    ident_dram = nc.inline_tensor(
                        np.eye(P, dtype=np.float32), name="ident_const"
                    )
                    ident = singles.tile([P, P], F32)
                    nc.gpsimd.dma_start(out=ident, in_=ident_dram.ap())
                    s_own_sb = singles.tile([P, IT, D], BF16)
                    nc.gpsimd.dma_start(out=s_own_sb, in_=s_own_q)
                elif sc == 1:
                    # s_perm on the FAST sync queue right behind chunk 1: it
                    # must land before the first s_out matmuls (~chunk 1's
                    # processing); the gpsimd software queue is far too slow
                    # for 560KB (observed landing at ~20us, stalling the PE)
                    nc.sync.dma_start(out=s_perm_sb, in_=s_perm_q)

            # ring of 3: the DVE adds run ahead of the PE so a PE hiccup
            # never starves the next chunk's add
            a_chs = [
                work.tile([P, IT, WJ], BF16, name=f"a_ch_{par}") for par in range(3)
            ]
            # aT[t] viewed two ways: f32 (evac), [512,2] bf16 (matmul rhs);
            # ring of 3 so the ACT evac of chunk jc+2 never collides with the
            # PE still reading chunk jc's tiles
            aTs = [
                [work.tile([P, WJ, 2], BF16, name=f"aT_{par}_{t}") for t in range(2)]
                for par in range(3)
            ]
            sin_sb = work.tile([D, N], BF16, name="sin_sb")
            sout_sb = work.tile([D, RB], BF16, name="sout_sb")

            psT = [
                [psT_pool.tile([P, RB], F32, name=f"psT_{par}_{t}") for t in range(2)]
                for par in range(2)
            ]
            psum_sins = [
                psSin_pool.tile([D, WJ], F32, name=f"psum_sin_{par}")
                for par in range(2)
            ]
            psum_sins.append(psWarm_pool.tile([D, WJ], F32, name="psum_sin_2"))
            psum_out = psOut_pool.tile([D, RB], F32)

            # ---- PE warm-up: defeat the HAM cold clock ----------------------
            # (writes garbage into sin bank 2, first really used at chunk 2)
            for _ in range(N_WARM):
                nc.tensor.matmul(
                    psum_sins[2], lhsT=wtile[:, :D], rhs=wtile[:, P:P + WJ],
                    start=True, stop=True,
                )

            def emit_sout_mms(jc, t):
                """s_out accumulation for chunk jc, pair-block t (aT already
                evacuated; runs one chunk behind the transposes)."""
                aT_b = aTs[jc % 3][t]
                for r in range(2):
                    k = (jc * 2 + t) * 2 + r
                    nc.tensor.matmul(
                        psum_out,
                        lhsT=s_perm_sb[:, k, :],
                        rhs=aT_b[:, :, r],
                        start=(k == 0),
                        stop=(k == 2 * 2 * JC - 1),
                    )

            def emit_filler(n):
                """Keep the PE's HAM activity window busy across early data
                gaps (idle > ~3.4us re-throttles the PE clock to 1.2 GHz)."""
                for _ in range(n):
                    nc.tensor.matmul(
                        psum_sins[2][:, :P], lhsT=wtile[:, :D], rhs=wtile[:, P:2 * P],
                        start=True, stop=True,
                    )

            def emit_sin_cast(jc, engine):
                """psum_sin -> sin_sb staging (on the scalar engine mid-stream:
                the DVE must stay add-only or the in-order DVE queue couples
                the adds to PE progress and stretches the whole pipeline),
                plus the quarter flush once both covered chunks are staged."""
                if engine == "scalar":
                    nc.scalar.copy(
                        out=sin_sb[:, jc * WJ:(jc + 1) * WJ], in_=psum_sins[jc % 3]
                    )
                else:
                    nc.vector.tensor_copy(
                        out=sin_sb[:, jc * WJ:(jc + 1) * WJ], in_=psum_sins[jc % 3]
                    )
                if jc % 2 == 1:
                    q = jc // 2
                    nc.sync.dma_start(
                        out=s_inT[q], in_=sin_sb[:, q * (N // 4):(q + 1) * (N // 4)]
                    )

            # ---- main loop over fp8 chunks ----------------------------------
            for jc in range(JC):
                par = jc % 3
                a_ch = a_chs[par]
                a_f32 = a_ch.bitcast(F32)  # [P, IT, WJ//2] packed bf16 pairs
                # channel add: fp8 planes in, bf16 out (upconvert on the DVE);
                # per-it pieces for the first/last chunk trim the chain
                # latency at the seams (Tile tracks subtile ranges)
                if jc == 0 or jc == JC - 1:
                    for it in range(IT):
                        nc.vector.tensor_add(
                            out=a_ch[:, it, :],
                            in0=raws[jc][:, 0, it, :],
                            in1=raws[jc][:, 1, it, :],
                        )
                else:
                    nc.vector.tensor_add(
                        out=a_ch, in0=raws[jc][:, 0], in1=raws[jc][:, 1]
                    )
                # previous chunk's s_out first: always ready (aT evacuated a
                # chunk ago), so the PE has work while this chunk's add lands
                if jc > 0:
                    emit_sout_mms(jc - 1, 0)
                    emit_sout_mms(jc - 1, 1)
                if jc == JC - 1:
                    # it-major so each transpose chases its add piece
                    for it in range(IT):
                        for t in range(2):
                            nc.tensor.transpose(
                                psT[jc % 2][t][:, it * P:(it + 1) * P],
                                a_f32[:, it, t * P:(t + 1) * P],
                                ident,
                            )
                    for t in range(2):
                        nc.scalar.copy(
                            out=aTs[jc % 3][t].bitcast(F32), in_=psT[jc % 2][t]
                        )
                else:
                    for t in range(2):
                        for it in range(IT):
                            nc.tensor.transpose(
                                psT[jc % 2][t][:, it * P:(it + 1) * P],
                                a_f32[:, it, t * P:(t + 1) * P],
                                ident,
                            )
                        nc.scalar.copy(
                            out=aTs[jc % 3][t].bitcast(F32), in_=psT[jc % 2][t]
                        )
                for it in range(IT):
                    nc.tensor.matmul(
                        psum_sins[jc % 3],
                        lhsT=s_own_sb[:, it, :],
                        rhs=a_ch[:, it, :],
                        start=(it == 0),
                        stop=(it == IT - 1),
                    )
                if jc > 0:
                    # after this chunk's aT evacs in the scalar queue, so the
                    # evacs (which gate next chunk's s_out) are never delayed
                    emit_sin_cast(jc - 1, "scalar")
                if jc < 2:
                    emit_filler(4)

            # ---- epilogue: last chunk's own s_out, casts, final flushes -----
            jc = JC - 1
            emit_sout_mms(jc, 0)
            emit_sout_mms(jc, 1)
            emit_sin_cast(jc, "vector")
            nc.scalar.copy(out=sout_sb, in_=psum_out)
            # last flush on the idle sync HWDGE queue, parallel to the gpsimd one
            nc.sync.dma_start(out=s_outT, in_=sout_sb)


_ENGINE_SEM_PREFIX = {
    "PE": "PE_",
    "DVE": "DVE_",
    "Activation": "Activation_",
    "Pool": "Pool_",
    "SP": "SP_",
}

_SKIP_OPS = ("InstEventSemaphore", "InstDrain", "InstDMACopy", "InstBranch")


def _strip_self_waits(nc: bass.Bass) -> int:
    """Drop semaphore waits where an instruction waits on its OWN engine's
    completion semaphore.  Engine queues issue and complete in order, so such
    waits are always runtime-satisfied; Tile emits them anyway and they push
    instructions past walrus codegen's per-opcode sync-wait limits (most
    compute encodings accept a single wait)."""
    stripped = 0
    for _, inst in nc.inst_map.items():
        if type(inst).__name__ in _SKIP_OPS:
            continue
        si = getattr(inst, "sync_info", None)
        if si is None or not si.on_wait:
            continue
        eng = getattr(inst, "engine", None)
        prefix = _ENGINE_SEM_PREFIX.get(getattr(eng, "name", ""), None)
        if prefix is None:
            continue
        kept = [w for w in si.on_wait if not w.ant_name.startswith(prefix)]
        if len(kept) != len(si.on_wait):
            stripped += len(si.on_wait) - len(kept)
            si.on_wait = kept
    return stripped


def _build() -> bass.Bass:
    nc = bacc.Bacc("TRN2", num_devices=NCORES)
    adjq = nc.dram_tensor("adjq", [JC, P, 2, IT, WJ], F8, kind="ExternalInput")
    s_own_q = nc.dram_tensor("s_own_q", [P, IT, D], BF16, kind="ExternalInput")
    s_perm_q = nc.dram_tensor("s_perm_q", [P, JC * 2 * 2, D], BF16, kind="ExternalInput")
    s_inT = [
        nc.dram_tensor(f"s_inT_{h}", [D, N // 4], BF16, kind="ExternalOutput")
        for h in range(4)
    ]
    s_outT = nc.dram_tensor("s_outT", [D, RB], BF16, kind="ExternalOutput")
    _emit(
        nc,
        adjq.ap(),
        s_own_q.ap(),
        s_perm_q.ap(),
        [t.ap() for t in s_inT],
        s_outT.ap(),
    )
    _strip_self_waits(nc)
    nc.finalize()
    return nc


_nc_cache = None


def _prep_core_inputs(adj_bf, s_bf, c):
    """Host-side relayout of core c's adjacency block and s tiles."""
    blk = adj_bf[c * RB:(c + 1) * RB]                    # (512, 4096, 2) fp8
    v = blk.reshape(IT, P, JC, WJ, 2)                     # it, p, jc, j, k
    v = v.transpose(2, 1, 4, 0, 3)                        # jc, p, k, it, j
    adjq = np.ascontiguousarray(v)
    s_own_q = np.ascontiguousarray(
        s_bf[c * RB:(c + 1) * RB].reshape(IT, P, D).transpose(1, 0, 2)
    )
    return {"adjq": adjq, "s_own_q": s_own_q}


def kernel(adj: np.ndarray, s: np.ndarray):
    global _nc_cache, LAST_RESULT
    adj = np.asarray(adj)
    s = np.asarray(s)
    assert adj.shape == (N, N, 2) and s.shape == (N, D)

    if _nc_cache is None:
        _nc_cache = _build()
    nc = _nc_cache

    # centered fp8 quantization: adj = (adj - 0.5) in e4m3 + exact rank-1
    # correction (sum_k 0.5 * colsum(s)) applied on the host after gather
    adj_bf = (np.asarray(adj, np.float32) - np.float32(0.5)).astype(
        ml_dtypes.float8_e4m3
    )
    s_bf = np.asarray(s, np.float32).astype(ml_dtypes.bfloat16)
    csum = np.asarray(s, np.float64).sum(axis=0)
    # s_perm[p, (jc, t, r)] = s[jc*512 + (t*128 + p)*2 + r]   (partition-major)
    s_perm = np.ascontiguousarray(
        s_bf.reshape(JC, 2, P, 2, D).transpose(2, 0, 1, 3, 4).reshape(P, JC * 4, D)
    )

    in_maps = []
    for c in range(NCORES):
        m = _prep_core_inputs(adj_bf, s_bf, c)
        m["s_perm_q"] = s_perm
        in_maps.append(m)

    res = bass_utils.run_bass_kernel_spmd(
        nc,
        in_maps,
        core_ids=list(range(NCORES)),
        trace=TRACE,
        **TRACE_KWARGS,
    )
    LAST_RESULT = res

    s_in = (
        (
            np.sum(
                [
                    np.concatenate(
                        [np.asarray(r[f"s_inT_{h}"], np.float32) for h in range(4)],
                        axis=1,
                    )
                    for r in res.results
                ],
                axis=0,
                dtype=np.float64,
            ).T
            + csum[None, :]
        )
        .astype(np.float32)
    )
    s_out = (
        np.concatenate(
            [np.asarray(r["s_outT"], np.float64).T for r in res.results], axis=0
        )
        + csum[None, :]
    ).astype(np.float32)
    return (np.ascontiguousarray(s_in), np.ascontiguousarray(s_out))
